# revision 1
# baseline (speedup 1.0000x reference)
"""Trainium2 Bass kernel for nn_AutoPruneNet (MLP policy/baseline heads + sampling).

Math (per row r of TB = T*B rows):
    h1 = relu(x @ W1.T + b1)            x: [512], h1: [400]
    h2 = relu(h1 @ W2.T + b2)           h2: [300]
    core = [h2, clip(reward,-1,1), last_action]   [302]
    pl = sigmoid(core @ Wp.T + bp)      [2]  (mu, sigma)
    baseline = core @ Wb.T + bb         [1]
    action = pl0 + pl1 * eps
    out[r] = [pl0, pl1, baseline, action]

Distribution: pure data parallel, TB rows split contiguously across 8 cores
(16384 rows each); weights replicated.

Device layout: activations stay feature-major ("transposed"): [feature, row],
so the contraction dim of every matmul sits on SBUF partitions and no on-chip
transposes are needed. The host pre-transposes the frame once and the output
back once. SBUF access patterns must start at partition 0/32/64/96, so:
  - the three head outputs are spread to psum partitions 0/32/64 via
    zero-padded head-weight columns, then moved to partition 0 by the ACT
    engine (which tolerates differing in/out partition bases);
  - [clip(reward); last_action] ride at partitions 96/97 of the last fc2
    output chunk (rows 44..95 zeroed), so the head contraction needs no
    extra matmul stream.
"""
import sys
import types

import numpy as np
import ml_dtypes

import concourse.bacc as bacc
import concourse.bass as bass
import concourse.mybir as mybir
import concourse.tile as tile
from concourse.bass import ds, ts
from concourse.bass_utils import run_bass_kernel_spmd


def _install_ntff_hook_shim():
    """Provide the optional antenv.axon_hooks module if the image lacks it,
    so a BASS_TRACE env var in the caller can't crash run_bass_kernel_spmd.
    Registers the real NTFF profile hook when the axon .so supports it."""
    try:
        import antenv.axon_hooks  # noqa: F401
        return
    except Exception:
        pass
    try:
        import antenv
    except Exception:
        return
    mod = types.ModuleType("antenv.axon_hooks")
    state = {"hook": None}
    mod.set_axon_ntff_profile_hook = lambda h: state.__setitem__("hook", h)
    mod.get_axon_ntff_profile_hook = lambda: state["hook"]
    sys.modules["antenv.axon_hooks"] = mod
    antenv.axon_hooks = mod
    try:
        from trn_agent_boot.trn_boot import _ntff_profile_via_ctypes
        mod.set_axon_ntff_profile_hook(
            _ntff_profile_via_ctypes('/opt/axon/libaxon_pjrt.so'))
    except Exception:
        pass


_install_ntff_hook_shim()

BF16 = ml_dtypes.bfloat16

N_CORES = 8
T, B, OBS = 64, 2048, 512
H1, H2 = 400, 300
TB = T * B
R = TB // N_CORES       # rows per core
NT = 512                # rows per row-tile (matmul moving dim)
OG = 4                  # row-tiles per output-DMA group

F32 = mybir.dt.float32
BF = mybir.dt.bfloat16
AF = mybir.ActivationFunctionType
ALU = mybir.AluOpType

# fc1 output (h1) chunking (also fc2 contraction chunking)
M1 = [(0, 100), (100, 100), (200, 100), (300, 100)]
# fc2 output (h2) chunking: {128, 128, 44}; chunk 2 padded to 98 rows with
# zeros at 44..95 and [cr; la] at 96..97
M2 = [(0, 128), (128, 128), (256, 44)]


def build_bass(rows: int):
    """Build the per-core Bass program for `rows` rows (rows % (NT*OG) == 0)."""
    assert rows % (NT * OG) == 0
    n_tiles = rows // NT

    nc = bacc.Bacc("TRN2", target_bir_lowering=False, debug=False)

    xt_d = nc.dram_tensor("xt", [128, 4, rows], BF, kind="ExternalInput")
    rwla_d = nc.dram_tensor("rwla", [2, rows], BF, kind="ExternalInput")
    eps_d = nc.dram_tensor("eps", [1, rows], F32, kind="ExternalInput")
    w1_d = nc.dram_tensor("w1", [128, 4, 400], BF, kind="ExternalInput")
    w2_d = nc.dram_tensor("w2", [100, 4, 300], BF, kind="ExternalInput")
    wh_d = nc.dram_tensor("wh", [128, 3, 65], BF, kind="ExternalInput")
    b1_d = nc.dram_tensor("b1", [100, 4], F32, kind="ExternalInput")
    b2_d = nc.dram_tensor("b2", [128, 3], F32, kind="ExternalInput")
    bh_d = nc.dram_tensor("bh", [65, 1], F32, kind="ExternalInput")
    out_d = nc.dram_tensor("out", [4, rows], F32, kind="ExternalOutput")

    with tile.TileContext(nc) as tc:
        with (
            tc.tile_pool(name="w", bufs=1) as wpool,
            tc.tile_pool(name="x", bufs=3) as xpool,
            tc.tile_pool(name="h1", bufs=8) as h1pool,
            tc.tile_pool(name="core", bufs=8) as cpool,
            tc.tile_pool(name="s", bufs=4) as spool,
            tc.tile_pool(name="ob", bufs=2) as opool,
            tc.tile_pool(name="ps1", bufs=4, space="PSUM") as ppool1,
            tc.tile_pool(name="ps2", bufs=2, space="PSUM") as ppool2,
            tc.tile_pool(name="ps3", bufs=2, space="PSUM") as ppool3,
        ):
            w1_sb = wpool.tile([128, 4, 400], BF, tag="w1")
            nc.scalar.dma_start(w1_sb[:], w1_d[:])
            w2_sb = wpool.tile([100, 4, 300], BF, tag="w2")
            nc.scalar.dma_start(w2_sb[:], w2_d[:])
            wh_sb = wpool.tile([128, 3, 65], BF, tag="wh")
            nc.scalar.dma_start(wh_sb[:], wh_d[:])
            b1_sb = wpool.tile([100, 4, 1], F32, tag="b1")
            nc.scalar.dma_start(b1_sb[:], b1_d[:])
            b2_sb = wpool.tile([128, 3, 1], F32, tag="b2")
            nc.scalar.dma_start(b2_sb[:], b2_d[:])
            bh_sb = wpool.tile([65, 1], F32, tag="bh")
            nc.scalar.dma_start(bh_sb[:], bh_d[:])

            # Software pipeline: the head matmuls + epilogue of tile t-1 are
            # emitted between fc1(t) and fc2(t), so fc2's matmuls get three
            # extra streams of slack for the fc1 relus to land (profiling
            # showed fc2 stalling ~1.2us on the relu semaphore otherwise).
            obs = {}        # group -> (ob tile, et8 tile)
            pending = None  # (cores, t) awaiting head + epilogue

            def emit_head_epilogue(cores, t):
                g, ti = divmod(t, OG)
                ob, et8 = obs[g]
                # heads: psum rows 0=mu_pre, 32=sigma_pre, 64=baseline_pre
                psh = ppool3.tile([65, NT], F32, tag="ps3")
                nc.tensor.matmul(psh[:], wh_sb[0:98, 2, :], cores[2][:],
                                 start=True, stop=False)
                nc.tensor.matmul(psh[:], wh_sb[:, 0, :], cores[0][:],
                                 start=False, stop=False)
                nc.tensor.matmul(psh[:], wh_sb[:, 1, :], cores[1][:],
                                 start=False, stop=True)
                # epilogue — ACT moves rows 32/64 down to partition 0
                sl = ds(ti * NT, NT)
                nc.scalar.activation(ob[:, 0, sl], psh[0:1, :],
                                     AF.Sigmoid, bias=bh_sb[0:1, :])
                nc.scalar.activation(ob[:, 1, sl], psh[32:33, :],
                                     AF.Sigmoid, bias=bh_sb[32:33, :])
                nc.scalar.activation(ob[:, 2, sl], psh[64:65, :],
                                     AF.Identity, bias=bh_sb[64:65, :])
                se = spool.tile([1, NT], F32, tag="se")
                nc.vector.tensor_mul(se[:], ob[:, 1, sl], et8[:, sl])
                nc.vector.tensor_add(ob[:, 3, sl], ob[:, 0, sl], se[:])
                if ti == OG - 1:
                    nc.sync.dma_start(out_d[0:4, ts(g, OG * NT)], ob[:])
                    del obs[g]

            for t in range(n_tiles + 1):
                h1s = None
                if t < n_tiles:
                    g = t // OG
                    if t % OG == 0:
                        # output buffer for this group of row-tiles
                        # (rows: pl0, pl1, baseline, action — at partition 0)
                        ob = opool.tile([1, 4, OG * NT], F32, tag="ob")
                        et8 = opool.tile([1, OG * NT], F32, tag="eps8")
                        nc.sync.dma_start(et8[:],
                                          eps_d[:, ts(g, OG * NT)])
                        obs[g] = (ob, et8)
                    xt_t = xpool.tile([128, 4, NT], BF, tag="xt")
                    nc.sync.dma_start(xt_t[:], xt_d[:, :, ts(t, NT)])

                    # fc1: h1T chunks of 100
                    h1s = []
                    for m, (m0, mw) in enumerate(M1):
                        ps = ppool1.tile([mw, NT], F32, tag="ps1")
                        for k in range(4):
                            nc.tensor.matmul(
                                ps[:],
                                w1_sb[:, k, ds(m0, mw)],
                                xt_t[:, k, :],
                                start=(k == 0),
                                stop=(k == 3),
                            )
                        hs = h1pool.tile([mw, NT], BF, tag=f"h1_{m}",
                                         name=f"h1_{m}")
                        # relu(psum + b1) on DVE: (in + bias) max 0
                        nc.vector.tensor_scalar(
                            hs[:], ps[:], b1_sb[0:mw, m, :], 0.0,
                            ALU.add, ALU.max
                        )
                        h1s.append(hs)

                if pending is not None:
                    emit_head_epilogue(*pending)
                    pending = None

                if t < n_tiles:
                    # fc2: h2T chunks {128, 128, 44+zeros+[cr;la]}; the m=2
                    # chunk goes first so its multi-engine assembly (relu +
                    # rwla DMA + clip + memsets) finishes before the head
                    # matmuls consume it
                    cores = [None, None, None]
                    for m in (2, 0, 1):
                        m0, mw = M2[m]
                        ps2 = ppool2.tile([mw, NT], F32, tag="ps2")
                        for k in range(4):
                            nc.tensor.matmul(
                                ps2[:],
                                w2_sb[0:M1[k][1], k, ds(m0, mw)],
                                h1s[k][:],
                                start=(k == 0),
                                stop=(k == 3),
                            )
                        if m < 2:
                            cm = cpool.tile([128, NT], BF, tag="c")
                            nc.scalar.activation(cm[:], ps2[:], AF.Relu,
                                                 bias=b2_sb[0:mw, m, :])
                        else:
                            cm = cpool.tile([98, NT], BF, tag="c2")
                            nc.gpsimd.memset(cm[32:64, :], 0.0)
                            nc.gpsimd.memset(cm[64:96, :], 0.0)
                            nc.scalar.activation(cm[0:mw, :], ps2[:], AF.Relu,
                                                 bias=b2_sb[0:mw, m, :])
                            nc.sync.dma_start(cm[96:98, :],
                                              rwla_d[:, ts(t, NT)])
                            nc.vector.tensor_scalar(
                                cm[96:97, :], cm[96:97, :], -1.0, 1.0,
                                ALU.max, ALU.min)
                        cores[m] = cm
                    pending = (cores, t)

    nc.compile()
    return nc


def host_prep(frame, reward, last_action, eps, W1, b1, W2, b2, Wp, bp, Wb, bb,
              rows=R, n_cores=N_CORES):
    """Shard + lay out inputs for the device program. Returns in_maps."""
    frame = np.asarray(frame, np.float32).reshape(TB, OBS)
    reward = np.asarray(reward, np.float32).reshape(TB)
    la = np.asarray(last_action).reshape(TB).astype(BF16)
    eps = np.asarray(eps, np.float32).reshape(TB)

    W1 = np.asarray(W1, np.float32)
    W2 = np.asarray(W2, np.float32)
    b1 = np.asarray(b1, np.float32)
    b2 = np.asarray(b2, np.float32)
    Wp = np.asarray(Wp, np.float32)
    bp = np.asarray(bp, np.float32)
    Wb = np.asarray(Wb, np.float32)
    bb = np.asarray(bb, np.float32)

    w1_h = np.ascontiguousarray(
        W1.T.reshape(4, 128, 400).transpose(1, 0, 2)).astype(BF16)
    w2_h = np.ascontiguousarray(
        W2.T.reshape(4, 100, 300).transpose(1, 0, 2)).astype(BF16)
    # head weights: columns 0/32/64 of a zero-padded 65-wide matrix hold
    # (mu, sigma, baseline); contraction rows follow the fc2 chunking
    # {128, 128, 44} with rows 44..95 zero and [cr; la] weights at 96/97
    Wh65 = np.zeros((302, 65), np.float32)
    Wh65[:, 0] = Wp[0]
    Wh65[:, 32] = Wp[1]
    Wh65[:, 64] = Wb[0]
    wh_h = np.zeros((128, 3, 65), np.float32)
    wh_h[:, 0, :] = Wh65[0:128]
    wh_h[:, 1, :] = Wh65[128:256]
    wh_h[0:44, 2, :] = Wh65[256:300]
    wh_h[96:98, 2, :] = Wh65[300:302]
    wh_h = wh_h.astype(BF16)
    b1_h = np.ascontiguousarray(b1.reshape(4, 100).T)
    b2_h = np.zeros((128, 3), np.float32)
    b2_h[0:128, 0] = b2[0:128]
    b2_h[0:128, 1] = b2[128:256]
    b2_h[0:44, 2] = b2[256:300]
    bh_h = np.zeros((65, 1), np.float32)
    bh_h[0, 0] = bp[0]
    bh_h[32, 0] = bp[1]
    bh_h[64, 0] = bb[0]

    in_maps = []
    for c in range(n_cores):
        sl = slice(c * rows, (c + 1) * rows)
        xt = np.ascontiguousarray(
            frame[sl].T.reshape(4, 128, rows).transpose(1, 0, 2)).astype(BF16)
        rwla = np.stack([reward[sl].astype(BF16), la[sl]], axis=0)
        in_maps.append({
            "xt": xt,
            "rwla": rwla,
            "eps": eps[sl].reshape(1, rows),
            "w1": w1_h, "w2": w2_h, "wh": wh_h,
            "b1": b1_h, "b2": b2_h, "bh": bh_h,
        })
    return in_maps


def assemble_out(per_core_outs):
    """[4, R] per core (rows: pl0, pl1, baseline, action) -> [T, B, 4]."""
    outs = []
    for o in per_core_outs:
        outs.append(np.asarray(o).T.reshape(-1, B, 4))
    return np.ascontiguousarray(
        np.concatenate(outs, axis=0).astype(np.float32))


_NC_CACHE = {}


def kernel(**inputs) -> np.ndarray:
    in_maps = host_prep(**inputs)
    if R not in _NC_CACHE:
        _NC_CACHE[R] = build_bass(R)
    nc = _NC_CACHE[R]
    res = run_bass_kernel_spmd(nc, in_maps, core_ids=list(range(N_CORES)))
    return assemble_out([res.results[c]["out"] for c in range(N_CORES)])



# revision 4
# speedup vs baseline: 1.4932x; 1.4932x over previous
"""Trainium2 Bass kernel for nn_AutoPruneNet (MLP policy/baseline heads + sampling).

Math (per row r of TB = T*B rows):
    h1 = relu(x @ W1.T + b1)            x: [512], h1: [400]
    h2 = relu(h1 @ W2.T + b2)           h2: [300]
    core = [h2, clip(reward,-1,1), last_action]   [302]
    pl = sigmoid(core @ Wp.T + bp)      [2]  (mu, sigma)
    baseline = core @ Wb.T + bb         [1]
    action = pl0 + pl1 * eps
    out[r] = [pl0, pl1, baseline, action]

Distribution: pure data parallel, TB rows split contiguously across 8 cores
(16384 rows each); weights replicated.

Device layout: activations stay feature-major ("transposed"): [feature, row],
so the contraction dim of every matmul sits on SBUF partitions and no on-chip
transposes are needed. The host pre-transposes the frame once and the output
back once.

fc1 and fc2 run in fp8e4 with MatmulPerfMode.DoubleRow (2 contraction rows
per PE cell -> 256-deep contraction per instruction), halving their matmul
stream count vs bf16. fp8e4 subnormal loss on the tiny MLP weights is avoided
by scaling W1 by S1=32 and W2 by S2/S1=32 on the host; the scales ride along
the activations (h1' = S1*h1 stored fp8, h2' = S2*h2 stored bf16) with the
biases pre-scaled and the head weights pre-divided by S2, so no extra device
ops are needed. Head matmuls stay bf16 for precision on the un-squashed
baseline channel.

Feature padding: h1 is padded 400->512 (zero W1 columns) so fc1 emits four
uniform 128-wide chunks forming two [128, 2, NT] fp8 DoubleRow pairs; fc2
output is padded 300->304 with chunks {128, 112, 64} so every stationary
free-dim is a multiple of 16 (DoubleRow AP stride constraint). The last fc2
chunk lands in a 98-partition tile with [clip(reward); last_action] at
partitions 96/97 (rows 64..95 zeroed), so the bf16 head contraction covers
all of core in 3 streams. Head outputs are spread to psum partitions 0/32/64
via zero-padded head-weight columns, then moved to partition 0 by the ACT
engine.
"""
import sys
import types

import numpy as np
import ml_dtypes

import concourse.bacc as bacc
import concourse.bass as bass
import concourse.mybir as mybir
import concourse.tile as tile
from concourse.bass import ds, ts
from concourse.bass_utils import run_bass_kernel_spmd


def _install_ntff_hook_shim():
    """Provide the optional antenv.axon_hooks module if the image lacks it,
    so a BASS_TRACE env var in the caller can't crash run_bass_kernel_spmd.
    Registers the real NTFF profile hook when the axon .so supports it."""
    try:
        import antenv.axon_hooks  # noqa: F401
        return
    except Exception:
        pass
    try:
        import antenv
    except Exception:
        return
    mod = types.ModuleType("antenv.axon_hooks")
    state = {"hook": None}
    mod.set_axon_ntff_profile_hook = lambda h: state.__setitem__("hook", h)
    mod.get_axon_ntff_profile_hook = lambda: state["hook"]
    sys.modules["antenv.axon_hooks"] = mod
    antenv.axon_hooks = mod
    try:
        from trn_agent_boot.trn_boot import _ntff_profile_via_ctypes
        mod.set_axon_ntff_profile_hook(
            _ntff_profile_via_ctypes('/opt/axon/libaxon_pjrt.so'))
    except Exception:
        pass


_install_ntff_hook_shim()

BF16 = ml_dtypes.bfloat16
FP8 = ml_dtypes.float8_e4m3   # TRN fp8e4 flavor (max +-240)

N_CORES = 8
T, B, OBS = 64, 2048, 512
H1, H2 = 400, 300
TB = T * B
R = TB // N_CORES       # rows per core
NT = 512                # rows per row-tile (matmul moving dim)
OG = 4                  # row-tiles per output-DMA group

S1 = 32.0               # fc1 weight scale (fp8 subnormal avoidance)
S2 = 1024.0             # cumulative scale on h2' = S2 * h2

F32 = mybir.dt.float32
BF = mybir.dt.bfloat16
F8 = mybir.dt.float8e4
AF = mybir.ActivationFunctionType
ALU = mybir.AluOpType
DR = mybir.MatmulPerfMode.DoubleRow

# fc2 output (h2) chunking: {128, 112, 64}; chunk 2 covers h2[240:304]
# (300:304 zero-padded), then zeros to 96 and [cr; la] at 96..97
M2 = [(0, 128), (128, 112), (240, 64)]


def build_bass(rows: int):
    """Build the per-core Bass program for `rows` rows (rows % (NT*OG) == 0)."""
    assert rows % (NT * OG) == 0
    n_tiles = rows // NT

    nc = bacc.Bacc("TRN2", target_bir_lowering=False, debug=False)

    xt_d = nc.dram_tensor("xt", [128, 4, rows], F8, kind="ExternalInput")
    rwla_d = nc.dram_tensor("rwla", [2, rows], BF, kind="ExternalInput")
    eps_d = nc.dram_tensor("eps", [1, rows], F32, kind="ExternalInput")
    w1_d = nc.dram_tensor("w1", [128, 4, 512], F8, kind="ExternalInput")
    w2_d = nc.dram_tensor("w2", [128, 4, 304], F8, kind="ExternalInput")
    wh_d = nc.dram_tensor("wh", [128, 3, 65], BF, kind="ExternalInput")
    b1_d = nc.dram_tensor("b1", [128, 4], F32, kind="ExternalInput")
    b2_d = nc.dram_tensor("b2", [128, 3], F32, kind="ExternalInput")
    bh_d = nc.dram_tensor("bh", [65, 1], F32, kind="ExternalInput")
    out_d = nc.dram_tensor("out", [4, rows], F32, kind="ExternalOutput")

    with tile.TileContext(nc) as tc:
        with (
            tc.tile_pool(name="w", bufs=1) as wpool,
            tc.tile_pool(name="x", bufs=3) as xpool,
            tc.tile_pool(name="h1", bufs=4) as h1pool,
            tc.tile_pool(name="core", bufs=8) as cpool,
            tc.tile_pool(name="s", bufs=4) as spool,
            tc.tile_pool(name="ob", bufs=2) as opool,
            tc.tile_pool(name="ps1", bufs=4, space="PSUM") as ppool1,
            tc.tile_pool(name="ps2", bufs=2, space="PSUM") as ppool2,
            tc.tile_pool(name="ps3", bufs=2, space="PSUM") as ppool3,
        ):
            w1_sb = wpool.tile([128, 4, 512], F8, tag="w1")
            nc.scalar.dma_start(w1_sb[:], w1_d[:])
            w2_sb = wpool.tile([128, 4, 304], F8, tag="w2")
            nc.scalar.dma_start(w2_sb[:], w2_d[:])
            wh_sb = wpool.tile([128, 3, 65], BF, tag="wh")
            nc.scalar.dma_start(wh_sb[:], wh_d[:])
            b1_sb = wpool.tile([128, 4, 1], F32, tag="b1")
            nc.scalar.dma_start(b1_sb[:], b1_d[:])
            b2_sb = wpool.tile([128, 3, 1], F32, tag="b2")
            nc.scalar.dma_start(b2_sb[:], b2_d[:])
            bh_sb = wpool.tile([65, 1], F32, tag="bh")
            nc.scalar.dma_start(bh_sb[:], bh_d[:])

            # Software pipeline: the head matmuls + epilogue of tile t-1 are
            # emitted between fc1(t) and fc2(t), so fc2's matmuls get slack
            # for the fc1 relus to land.
            obs = {}        # group -> (ob tile, et8 tile)
            pending = None  # (cores, t) awaiting head + epilogue

            def emit_head_epilogue(cores, t):
                g, ti = divmod(t, OG)
                ob, et8 = obs[g]
                # heads: psum rows 0=mu_pre, 32=sigma_pre, 64=baseline_pre
                psh = ppool3.tile([65, NT], F32, tag="ps3")
                nc.tensor.matmul(psh[:], wh_sb[0:98, 2, :], cores[2][:],
                                 start=True, stop=False)
                nc.tensor.matmul(psh[:], wh_sb[0:128, 0, :], cores[0][:],
                                 start=False, stop=False)
                nc.tensor.matmul(psh[:], wh_sb[0:112, 1, :],
                                 cores[1][0:112, :],
                                 start=False, stop=True)
                # epilogue — ACT moves rows 32/64 down to partition 0
                sl = ds(ti * NT, NT)
                nc.scalar.activation(ob[:, 0, sl], psh[0:1, :],
                                     AF.Sigmoid, bias=bh_sb[0:1, :])
                nc.scalar.activation(ob[:, 1, sl], psh[32:33, :],
                                     AF.Sigmoid, bias=bh_sb[32:33, :])
                nc.scalar.activation(ob[:, 2, sl], psh[64:65, :],
                                     AF.Identity, bias=bh_sb[64:65, :])
                se = spool.tile([1, NT], F32, tag="se")
                nc.vector.tensor_mul(se[:], ob[:, 1, sl], et8[:, sl])
                nc.vector.tensor_add(ob[:, 3, sl], ob[:, 0, sl], se[:])
                if ti == OG - 1:
                    nc.sync.dma_start(out_d[0:4, ts(g, OG * NT)], ob[:])
                    del obs[g]

            for t in range(n_tiles + 1):
                h1s = None
                if t < n_tiles:
                    g = t // OG
                    if t % OG == 0:
                        # output buffer for this group of row-tiles
                        # (rows: pl0, pl1, baseline, action — at partition 0)
                        ob = opool.tile([1, 4, OG * NT], F32, tag="ob")
                        et8 = opool.tile([1, OG * NT], F32, tag="eps8")
                        nc.sync.dma_start(et8[:],
                                          eps_d[:, ts(g, OG * NT)])
                        obs[g] = (ob, et8)
                    xt_t = xpool.tile([128, 4, NT], F8, tag="xt")
                    nc.sync.dma_start(xt_t[:], xt_d[:, :, ts(t, NT)])

                    # fc1: h1'T = S1*relu(h1_pre) in two fp8 DoubleRow pair
                    # tiles [128, 2, NT]: hA = feats 0:256, hB = 256:512
                    # (400:512 zero via zero-padded W1 columns)
                    hA = h1pool.tile([128, 2, NT], F8, tag="h1a")
                    hB = h1pool.tile([128, 2, NT], F8, tag="h1b")
                    h1s = (hA, hB)
                    for m in range(4):
                        ps = ppool1.tile([128, NT], F32, tag="ps1")
                        nc.tensor.matmul(
                            ps[:], w1_sb[:, 0:2, ds(m * 128, 128)],
                            xt_t[:, 0:2, :],
                            start=True, stop=False, perf_mode=DR)
                        nc.tensor.matmul(
                            ps[:], w1_sb[:, 2:4, ds(m * 128, 128)],
                            xt_t[:, 2:4, :],
                            start=False, stop=True, perf_mode=DR)
                        ht = (hA, hB)[m // 2]
                        # relu(psum + S1*b1) on DVE: (in + bias) max 0
                        nc.vector.tensor_scalar(
                            ht[:, m % 2, :], ps[:], b1_sb[:, m, :], 0.0,
                            ALU.add, ALU.max
                        )

                if pending is not None:
                    emit_head_epilogue(*pending)
                    pending = None

                if t < n_tiles:
                    # fc2: h2'T = S2*relu(h2_pre), chunks {128, 112, 64}; the
                    # m=2 chunk goes first so its multi-engine assembly (relu
                    # + rwla DMA + clip + memset) finishes before the head
                    # matmuls consume it
                    hA, hB = h1s
                    cores = [None, None, None]
                    for m in (2, 0, 1):
                        m0, mw = M2[m]
                        ps2 = ppool2.tile([mw, NT], F32, tag="ps2")
                        nc.tensor.matmul(
                            ps2[:], w2_sb[:, 0:2, ds(m0, mw)], hA[:],
                            start=True, stop=False, perf_mode=DR)
                        nc.tensor.matmul(
                            ps2[:], w2_sb[:, 2:4, ds(m0, mw)], hB[:],
                            start=False, stop=True, perf_mode=DR)
                        if m < 2:
                            cm = cpool.tile([128, NT], BF, tag="c")
                            nc.scalar.activation(cm[0:mw, :], ps2[:], AF.Relu,
                                                 bias=b2_sb[0:mw, m, :])
                        else:
                            cm = cpool.tile([98, NT], BF, tag="c2")
                            nc.gpsimd.memset(cm[64:96, :], 0.0)
                            nc.scalar.activation(cm[0:mw, :], ps2[:], AF.Relu,
                                                 bias=b2_sb[0:mw, m, :])
                            nc.sync.dma_start(cm[96:98, :],
                                              rwla_d[:, ts(t, NT)])
                            nc.vector.tensor_scalar(
                                cm[96:97, :], cm[96:97, :], -1.0, 1.0,
                                ALU.max, ALU.min)
                        cores[m] = cm
                    pending = (cores, t)

    nc.compile()
    return nc


def host_prep(frame, reward, last_action, eps, W1, b1, W2, b2, Wp, bp, Wb, bb,
              rows=R, n_cores=N_CORES):
    """Shard + lay out inputs for the device program. Returns in_maps."""
    frame = np.asarray(frame, np.float32).reshape(TB, OBS)
    reward = np.asarray(reward, np.float32).reshape(TB)
    la = np.asarray(last_action).reshape(TB).astype(BF16)
    eps = np.asarray(eps, np.float32).reshape(TB)

    W1 = np.asarray(W1, np.float32)
    W2 = np.asarray(W2, np.float32)
    b1 = np.asarray(b1, np.float32)
    b2 = np.asarray(b2, np.float32)
    Wp = np.asarray(Wp, np.float32)
    bp = np.asarray(bp, np.float32)
    Wb = np.asarray(Wb, np.float32)
    bb = np.asarray(bb, np.float32)

    # W1T scaled by S1, padded 400 -> 512 output features, fp8:
    # w1[p, b, o] = S1 * W1[o, 128b+p]
    w1t = np.zeros((512, 512), np.float32)
    w1t[:, 0:400] = S1 * W1.T
    w1_h = np.ascontiguousarray(
        w1t.reshape(4, 128, 512).transpose(1, 0, 2)).astype(FP8)
    # W2T scaled by S2/S1, padded [400->512, 300->304], fp8
    w2t = np.zeros((512, 304), np.float32)
    w2t[0:400, 0:300] = (S2 / S1) * W2.T
    w2_h = np.ascontiguousarray(
        w2t.reshape(4, 128, 304).transpose(1, 0, 2)).astype(FP8)
    # head weights: columns 0/32/64 of a zero-padded 65-wide matrix hold
    # (mu, sigma, baseline); contraction rows follow the fc2 chunking
    # {128, 112, 64-pad} with h2 rows divided by S2 and [cr; la] weights
    # (unscaled) at rows 96/97 of the third plane
    Wh65 = np.zeros((302, 65), np.float32)
    Wh65[:, 0] = Wp[0]
    Wh65[:, 32] = Wp[1]
    Wh65[:, 64] = Wb[0]
    wh_h = np.zeros((128, 3, 65), np.float32)
    wh_h[0:128, 0, :] = Wh65[0:128] / S2
    wh_h[0:112, 1, :] = Wh65[128:240] / S2
    wh_h[0:60, 2, :] = Wh65[240:300] / S2
    wh_h[96:98, 2, :] = Wh65[300:302]
    wh_h = wh_h.astype(BF16)
    b1p = np.zeros(512, np.float32)
    b1p[0:400] = S1 * b1
    b1_h = np.ascontiguousarray(b1p.reshape(4, 128).T)
    b2_h = np.zeros((128, 3), np.float32)
    b2_h[0:128, 0] = S2 * b2[0:128]
    b2_h[0:112, 1] = S2 * b2[128:240]
    b2_h[0:60, 2] = S2 * b2[240:300]
    bh_h = np.zeros((65, 1), np.float32)
    bh_h[0, 0] = bp[0]
    bh_h[32, 0] = bp[1]
    bh_h[64, 0] = bb[0]

    in_maps = []
    for c in range(n_cores):
        sl = slice(c * rows, (c + 1) * rows)
        xt = np.ascontiguousarray(
            frame[sl].T.reshape(4, 128, rows).transpose(1, 0, 2)).astype(FP8)
        rwla = np.stack([reward[sl].astype(BF16), la[sl]], axis=0)
        in_maps.append({
            "xt": xt,
            "rwla": rwla,
            "eps": eps[sl].reshape(1, rows),
            "w1": w1_h, "w2": w2_h, "wh": wh_h,
            "b1": b1_h, "b2": b2_h, "bh": bh_h,
        })
    return in_maps


def assemble_out(per_core_outs):
    """[4, R] per core (rows: pl0, pl1, baseline, action) -> [T, B, 4]."""
    outs = []
    for o in per_core_outs:
        outs.append(np.asarray(o).T.reshape(-1, B, 4))
    return np.ascontiguousarray(
        np.concatenate(outs, axis=0).astype(np.float32))


_NC_CACHE = {}


def kernel(**inputs) -> np.ndarray:
    in_maps = host_prep(**inputs)
    if R not in _NC_CACHE:
        _NC_CACHE[R] = build_bass(R)
    nc = _NC_CACHE[R]
    res = run_bass_kernel_spmd(nc, in_maps, core_ids=list(range(N_CORES)))
    return assemble_out([res.results[c]["out"] for c in range(N_CORES)])
